# revision 8
# baseline (speedup 1.0000x reference)
"""Trainium2 Bass kernel for nn_DfOpCoefLoop (deep-filter complex FIR + alpha blend).

Reference semantics (per batch b, time t, freq bin f < 96):
    spec_f[t,f] = sum_{i=0..4} x[t+i-2, f] * coefs[t,i,f]      (complex MAC, zero-padded in t)
    out[t,f]    = alpha[t] * spec_f[t,f] + (1-alpha[t]) * x[t,f]
    out[t,f]    = spec[t,f]                                    (f >= 96 passthrough)

Strategy: pure data-parallel over batch (32 batches -> 8 cores x 4).
Per core, time is tiled 128-per-partition-chunk. Shifted windows are built with
cheap SBUF->SBUF row-copy DMAs from a per-batch staging block. The complex MAC
is two big strided multiplies + strided tensor_reduce:
    m1  = [xr*cr | xi*(-ci)]  -> full reduce over (comp,tap) -> re
    m2a = xi*cr, m2b = xr*(-ci) -> reduce (negate on b) -> im
with coefficients shipped from host pre-arranged as [cr | -ci] planar, so no
on-device negation/transposition is needed. The alpha blend is fused into one
scalar_tensor_tensor op with per-partition alpha; (1-alpha)*x0 runs on the
Scalar engine. m1 alternates DVE/GPSIMD per chunk to balance engine load.
The f>=96 passthrough is a DRAM->DRAM DMA that never touches SBUF.
"""

import numpy as np

ORDER = 5
LOOKAHEAD = 2
F = 96            # deep-filtered bins
FC = 2 * F        # interleaved (f, re/im) row: 192 floats
W = ORDER * FC    # 960: stacked shifted windows / coef row
NFREQ = 481
REST = (NFREQ - F) * 2  # 770 passthrough floats per row
ROW = NFREQ * 2   # 962
B, T = 32, 1000
NCORES = 8
BPC = B // NCORES  # batches per core

_CACHE = {}


def _build_program(bpc, t_len):
    """Build the per-core Bass program. Returns nc (ready for run_bass_kernel_spmd)."""
    import concourse.bacc as bacc
    import concourse.mybir as mybir
    import concourse.tile as tile

    nk = (t_len + 127) // 128          # time chunks per batch
    ncols = bpc * nk                   # alpha table columns

    # Bacc (not raw Bass): its compile() runs generate_event_semaphores,
    # which splits multi-wait sync onto EventSemaphore instructions --
    # TRN2 instructions encode at most one sem wait.
    nc = bacc.Bacc("TRN2", target_bir_lowering=False, debug=False)
    dt = mybir.dt.float32

    spec_df = nc.dram_tensor("spec_df", [bpc, t_len, FC], dt, kind="ExternalInput").ap()
    spec_rest = nc.dram_tensor("spec_rest", [bpc, t_len, REST], dt, kind="ExternalInput").ap()
    coefs_x = nc.dram_tensor("coefs_x", [bpc, t_len, W], dt, kind="ExternalInput").ap()
    alpha_t = nc.dram_tensor("alpha_t", [128, ncols], dt, kind="ExternalInput").ap()
    oma_t = nc.dram_tensor("oma_t", [128, ncols], dt, kind="ExternalInput").ap()
    out = nc.dram_tensor("out", [bpc, t_len, ROW], dt, kind="ExternalOutput").ap()

    mul = mybir.AluOpType.mult
    add = mybir.AluOpType.add
    copy_fn = mybir.ActivationFunctionType.Copy

    def rows_of(k):
        return min(128, t_len - 128 * k)

    with tile.TileContext(nc) as tc:
        with (
            tc.tile_pool(name="const", bufs=1) as const_pool,
            tc.tile_pool(name="xb", bufs=2) as xb_pool,
            tc.tile_pool(name="cxb", bufs=2) as cxb_pool,
            tc.tile_pool(name="ob", bufs=2) as ob_pool,
            tc.tile_pool(name="x5", bufs=3) as x5_pool,
            tc.tile_pool(name="prod", bufs=2) as prod_pool,
            tc.tile_pool(name="small", bufs=3) as small_pool,
        ):
            alpha_sb = const_pool.tile([128, ncols], dt, name="alpha_sb")
            oma_sb = const_pool.tile([128, ncols], dt, name="oma_sb")
            nc.sync.dma_start(alpha_sb[:], alpha_t[:])
            nc.sync.dma_start(oma_sb[:], oma_t[:])

            for b in range(bpc):
                # f>=96 passthrough: DRAM->DRAM, no SBUF involvement
                nc.sync.dma_start(out[b, :, FC:ROW], spec_rest[b])

                # Stage this batch's DF bins and coefs as (partition=t%128, chunk, row)
                xb = xb_pool.tile([128, nk * FC], dt, name="xb")
                cxb = cxb_pool.tile([128, nk * W], dt, name="cxb")
                ob = ob_pool.tile([128, nk * FC], dt, name="ob")

                nfull = t_len // 128
                rem = t_len - 128 * nfull
                if nfull:
                    nc.sync.dma_start(
                        xb[:, 0 : nfull * FC].rearrange("p (k f) -> p k f", k=nfull),
                        spec_df[b, 0 : 128 * nfull].rearrange("(k p) f -> p k f", p=128),
                    )
                    nc.sync.dma_start(
                        cxb[:, 0 : nfull * W].rearrange("p (k f) -> p k f", k=nfull),
                        coefs_x[b, 0 : 128 * nfull].rearrange("(k p) f -> p k f", p=128),
                    )
                if rem:
                    nc.sync.dma_start(
                        xb[0:rem, nfull * FC : (nfull + 1) * FC],
                        spec_df[b, 128 * nfull : t_len],
                    )
                    nc.sync.dma_start(
                        cxb[0:rem, nfull * W : (nfull + 1) * W],
                        coefs_x[b, 128 * nfull : t_len],
                    )

                for k in range(nk):
                    rows = rows_of(k)
                    x5 = x5_pool.tile([128, W], dt, name="x5")

                    # Build the 5 shifted windows: x5[p, i*FC:...] = x[128k+p+i-2]
                    for i in range(ORDER):
                        d = i - LOOKAHEAD
                        dsl = slice(i * FC, (i + 1) * FC)
                        # main part from chunk k
                        lo = max(0, -d)
                        hi = min(rows, rows_of(k) - d) if d > 0 else rows
                        # boundary rows that need zeros (compute ops must start
                        # at partition 0/32/64/96, so memset the whole slice
                        # first and let the main DMA overwrite the valid rows)
                        needs_zero = (d < 0 and k == 0) or (
                            d > 0 and hi < rows and 128 * k + hi + d >= t_len
                        )
                        if needs_zero:
                            nc.vector.memset(x5[:, dsl], 0.0)
                        if hi > lo:
                            nc.sync.dma_start(
                                x5[lo:hi, dsl],
                                xb[lo + d : hi + d, k * FC : (k + 1) * FC],
                            )
                        if d < 0 and k > 0:
                            # dst p in [0, -d): chunk k-1 partitions 128+d..
                            nc.sync.dma_start(
                                x5[0:-d, dsl],
                                xb[128 + d : 128, (k - 1) * FC : k * FC],
                            )
                        elif d > 0 and hi < rows and 128 * k + hi + d < t_len:
                            # dst p in [hi, rows): chunk k+1 partitions 0..
                            nc.sync.dma_start(
                                x5[hi:rows, dsl],
                                xb[0 : rows - hi, (k + 1) * FC : (k + 2) * FC],
                            )

                    cx = cxb[0:rows, k * W : (k + 1) * W]
                    x5r = x5[0:rows]
                    xv = x5r.rearrange("p (i f c) -> p c i f", i=ORDER, f=F, c=2)
                    xr = xv[:, 0:1].squeeze(1)   # (rows, 5, 96)
                    xi = xv[:, 1:2].squeeze(1)
                    cv = cx.rearrange("p (g i f) -> p g i f", g=2, i=ORDER, f=F)
                    cr = cv[:, 0:1].squeeze(1)
                    mci = cv[:, 1:2].squeeze(1)

                    p1 = prod_pool.tile([128, W], dt, name="p1")
                    p2 = prod_pool.tile([128, W], dt, name="p2")
                    acc = small_pool.tile([128, FC], dt, name="acc")
                    sa = small_pool.tile([128, F], dt, name="sa")
                    sb_t = small_pool.tile([128, F], dt, name="sb_t")
                    v = small_pool.tile([128, FC], dt, name="v")

                    # m1 -> P1 = [xr*cr | xi*(-ci)] ; alternate engine for balance
                    m1_eng = nc.vector if (k % 2 == 0) else nc.gpsimd
                    m1_eng.tensor_mul(
                        p1[0:rows].rearrange("p (g i f) -> p g i f", g=2, i=ORDER, f=F),
                        xv,
                        cv,
                    )
                    # r1: full reduce over (comp, tap) -> re
                    nc.vector.tensor_reduce(
                        acc[0:rows, 0:F],
                        p1[0:rows].rearrange("p (gi f) -> p f gi", gi=2 * ORDER, f=F),
                        axis=mybir.AxisListType.X,
                        op=add,
                    )
                    # m2a = xi*cr ; m2b = xr*(-ci)   (GPSIMD)
                    nc.gpsimd.tensor_mul(
                        p2[0:rows, 0 : ORDER * F].rearrange("p (i f) -> p i f", i=ORDER, f=F),
                        xi,
                        cr,
                    )
                    nc.gpsimd.tensor_mul(
                        p2[0:rows, ORDER * F : W].rearrange("p (i f) -> p i f", i=ORDER, f=F),
                        xr,
                        mci,
                    )
                    nc.vector.tensor_reduce(
                        sa[0:rows],
                        p2[0:rows, 0 : ORDER * F].rearrange("p (i f) -> p f i", i=ORDER, f=F),
                        axis=mybir.AxisListType.X,
                        op=add,
                    )
                    nc.vector.tensor_reduce(
                        sb_t[0:rows],
                        p2[0:rows, ORDER * F : W].rearrange("p (i f) -> p f i", i=ORDER, f=F),
                        axis=mybir.AxisListType.X,
                        op=add,
                        negate=True,
                    )
                    # im = sa + sb
                    nc.vector.tensor_add(acc[0:rows, F:FC], sa[0:rows], sb_t[0:rows])

                    col = b * nk + k
                    # v = (1-alpha) * x0   (x0 = d=0 slice of x5, deinterleaved view)
                    nc.scalar.activation(
                        v[0:rows].rearrange("p (c f) -> p c f", c=2, f=F),
                        x5[0:rows, LOOKAHEAD * FC : (LOOKAHEAD + 1) * FC].rearrange(
                            "p (f c) -> p c f", f=F, c=2
                        ),
                        copy_fn,
                        scale=oma_sb[0:rows, col : col + 1],
                    )
                    # out = alpha*acc + v, written interleaved into the out block
                    nc.vector.scalar_tensor_tensor(
                        ob[0:rows, k * FC : (k + 1) * FC].rearrange(
                            "p (f c) -> p c f", f=F, c=2
                        ),
                        acc[0:rows].rearrange("p (c f) -> p c f", c=2, f=F),
                        alpha_sb[0:rows, col : col + 1],
                        v[0:rows].rearrange("p (c f) -> p c f", c=2, f=F),
                        op0=mul,
                        op1=add,
                    )

                # store the computed DF bins
                if nfull:
                    nc.sync.dma_start(
                        out[b, 0 : 128 * nfull, 0:FC].rearrange("(k p) f -> p k f", p=128),
                        ob[:, 0 : nfull * FC].rearrange("p (k f) -> p k f", k=nfull),
                    )
                if rem:
                    nc.sync.dma_start(
                        out[b, 128 * nfull : t_len, 0:FC],
                        ob[0:rem, nfull * FC : (nfull + 1) * FC],
                    )
    nc.compile()
    return nc


def _get_program(bpc=BPC, t_len=T):
    key = (bpc, t_len)
    if key not in _CACHE:
        _CACHE[key] = _build_program(bpc, t_len)
    return _CACHE[key]


def _host_prep(spec, coefs, alpha, bpc, t_len):
    """Slice + re-layout one core's inputs. Returns the in_map dict."""
    nk = (t_len + 127) // 128
    b = spec.shape[0]
    assert b == bpc
    spec2 = np.ascontiguousarray(spec[:, 0], dtype=np.float32)  # (bpc, t, 481, 2)
    spec_df = np.ascontiguousarray(spec2[:, :, :F, :]).reshape(bpc, t_len, FC)
    spec_rest = np.ascontiguousarray(spec2[:, :, F:, :]).reshape(bpc, t_len, REST)
    cr = np.ascontiguousarray(coefs[..., 0], dtype=np.float32).reshape(bpc, t_len, ORDER * F)
    ci = np.ascontiguousarray(coefs[..., 1], dtype=np.float32).reshape(bpc, t_len, ORDER * F)
    coefs_x = np.concatenate([cr, -ci], axis=-1)  # (bpc, t, 960)

    al = np.zeros((bpc, nk * 128), np.float32)
    al[:, :t_len] = alpha[:, :, 0]
    # col = b*nk + k holds alpha[b, 128k + p] at partition p
    alpha_t = np.ascontiguousarray(
        al.reshape(bpc, nk, 128).transpose(2, 0, 1).reshape(128, bpc * nk)
    )
    oma_t = np.ascontiguousarray(1.0 - alpha_t)
    return {
        "spec_df": spec_df,
        "spec_rest": spec_rest,
        "coefs_x": coefs_x,
        "alpha_t": alpha_t,
        "oma_t": oma_t,
    }


def run_on_cores(spec, coefs, alpha, trace=False):
    """Full-input entry: shard, run on 8 cores, return (out_full, results_obj)."""
    from concourse import bass_utils

    nc = _get_program()
    in_maps = [
        _host_prep(
            spec[c * BPC : (c + 1) * BPC],
            coefs[c * BPC : (c + 1) * BPC],
            alpha[c * BPC : (c + 1) * BPC],
            BPC,
            T,
        )
        for c in range(NCORES)
    ]
    res = bass_utils.run_bass_kernel_spmd(
        nc, in_maps, core_ids=list(range(NCORES)), trace=trace
    )
    outs = [res.results[c]["out"].reshape(BPC, 1, T, NFREQ, 2) for c in range(NCORES)]
    full = np.concatenate(outs, axis=0).astype(np.float32)
    return full, res


def kernel(spec, coefs, alpha):
    spec = np.asarray(spec, dtype=np.float32)
    coefs = np.asarray(coefs, dtype=np.float32)
    alpha = np.asarray(alpha, dtype=np.float32)
    full, _ = run_on_cores(spec, coefs, alpha, trace=False)
    return full
